# revision 5
# baseline (speedup 1.0000x reference)
"""GATv2 3-layer kernel for 8 TRN2 NeuronCores (Bass/Tile).

Dst-sharded: each core owns 12500 dst nodes, replicates the tiny dense
transforms for all nodes into a local DRAM gather table, then runs a
dst-major edge phase (dma_gather of per-edge source rows, DVE softmax +
weighted sum), PE-transposes layer outputs and AllGathers them between
layers.

Perf structure:
- The per-edge table gather is SWDGE (gpsimd Q7 descriptor generation)
  bound; the 4 chunk-gathers per block are striped across the 4 SWDGE
  queues (queue q runs on Q7 core pair q) which pipelines descriptor
  generation ~3x.
- Nodes are grouped into 128-row blocks by max-per-chunk degree so the
  per-(block,chunk) padded width (max over 8 cores x 128 partitions)
  stays near the mean.
- Dense transforms batch 7 node-tiles per PE matmul using host-built
  block-diagonal weights ([7*din, 7*64]); one LDWEIGHTS+MATMUL per 896
  nodes instead of 7.
- Padded slots point at per-chunk magic rows (+-1000) so exp -> 0; att
  is folded into the weights (u = |att|*(xl+xr)) with a sign-split
  min/max leaky-relu.
"""
import sys

sys.path.insert(0, "/opt/trn_rl_repo")

import numpy as np

N = 100000
NCORES = 8
SH = 12500
PSH = 12544                 # 98 * 128
NBLK = 98
NN = NCORES * PSH           # 100352
CSTRIDE = 25089             # chunk stride in table rows (incl magic row)
CNODES = 25088              # real rows per chunk (2 core shards)
MAGIC_LOCAL = CNODES
NCHUNK = 4
NTAB = NCHUNK * CSTRIDE
DIMS = [(11, 16), (16, 32), (32, 64)]
GRP = 896                   # dense-phase node group (7*128)
NGRP = PSH // GRP           # 14

TRACE = False
LAST_EXEC_NS = None


def _preprocess(edge_index):
    src = np.concatenate([edge_index[0].astype(np.int64), np.arange(N, dtype=np.int64)])
    dst = np.concatenate([edge_index[1].astype(np.int64), np.arange(N, dtype=np.int64)])
    node_owner = np.arange(N) // SH
    node_chunk = node_owner // 2

    cnt = np.zeros((N, NCHUNK), dtype=np.int32)
    np.add.at(cnt, (dst, node_chunk[src]), 1)

    localpos = np.empty(N, dtype=np.int64)
    order_per_core = []
    for c in range(NCORES):
        nodes = np.arange(c * SH, (c + 1) * SH)
        cc = cnt[nodes]
        o = np.lexsort((cc.sum(1), cc.max(1)))[::-1]
        nodes = nodes[o]
        order_per_core.append(nodes)
        localpos[nodes] = np.arange(SH)

    tabrow = (node_owner // 2) * CSTRIDE + (node_owner % 2) * PSH + localpos

    cntp = np.zeros((NCORES, NBLK, 128, NCHUNK), dtype=np.int32)
    for c in range(NCORES):
        cc = cnt[order_per_core[c]]
        cc = np.concatenate([cc, np.zeros((PSH - SH, NCHUNK), np.int32)], 0)
        cntp[c] = cc.reshape(NBLK, 128, NCHUNK)
    D = cntp.max(axis=(0, 2)).astype(np.int64)      # [NBLK, NCHUNK]
    Dtot = D.sum(axis=1)                            # [NBLK]
    SDT = int(Dtot.sum())

    blk_base = np.r_[0, np.cumsum(Dtot)][:-1]
    coloff = np.zeros((NBLK, NCHUNK), dtype=np.int64)
    for b in range(NBLK):
        coloff[b] = blk_base[b] + np.r_[0, np.cumsum(D[b])][:-1]

    # slot grid [core, 128, SDT], value = chunk-local table row of src
    ecore = dst // SH
    edl = localpos[dst]
    eblk, epart = edl // 128, edl % 128
    echunk = node_chunk[src]
    eval_loc = tabrow[src] - echunk * CSTRIDE
    key = ((ecore * NBLK + eblk) * 128 + epart) * NCHUNK + echunk
    eo = np.argsort(key, kind='stable')
    keys, vals = key[eo], eval_loc[eo]
    grp_start = np.r_[0, np.flatnonzero(np.diff(keys)) + 1]
    grp_len = np.diff(np.r_[grp_start, len(keys)])
    jidx = np.arange(len(keys)) - np.repeat(grp_start, grp_len)
    kc = keys % NCHUNK
    kp = (keys // NCHUNK) % 128
    kb = (keys // (NCHUNK * 128)) % NBLK
    kcore = keys // (NCHUNK * 128 * NBLK)
    slots = np.full((NCORES, 128, SDT), MAGIC_LOCAL, dtype=np.int64)
    slots[kcore, kp, coloff[kb, kc] + jidx] = vals

    # wrapped int16 per (block, chunk) call, concatenated along free dim
    idx_flat = np.empty((NCORES, 128, 8 * SDT), dtype=np.int16)
    for b in range(NBLK):
        for ch in range(NCHUNK):
            w = int(D[b, ch])
            if w == 0:
                continue
            cs = int(coloff[b, ch])
            sub = slots[:, :, cs:cs + w]                          # [NC,128,w]
            lst = sub.transpose(0, 2, 1).reshape(NCORES, w * 128)  # pos=j*128+p
            wr = lst.reshape(NCORES, w * 8, 16).transpose(0, 2, 1)
            wr = np.tile(wr, (1, 8, 1))
            idx_flat[:, :, 8 * cs:8 * (cs + w)] = wr.astype(np.int16)

    meta = dict(D=D, Dtot=Dtot, coloff=coloff, blk_base=blk_base,
                order_per_core=order_per_core, SDT=SDT)
    return idx_flat, meta


def _build_program(meta, kpos_list):
    import concourse.bass as bass
    import concourse.bacc as bacc
    import concourse.tile as tile
    import concourse.mybir as mybir
    from concourse import masks

    D, Dtot, coloff = meta['D'], meta['Dtot'], meta['coloff']
    blk_base, SDT = meta['blk_base'], meta['SDT']
    f32 = mybir.dt.float32
    i16 = mybir.dt.int16
    AF = mybir.ActivationFunctionType
    OP = mybir.AluOpType
    AX = mybir.AxisListType

    nc = bacc.Bacc("TRN2", target_bir_lowering=False, debug=False,
                   num_devices=NCORES, num_swdge_queues=4)
    t_xT = nc.dram_tensor("xT", [11, NN], f32, kind="ExternalInput")
    t_xTloc = nc.dram_tensor("xTloc", [11, PSH], f32, kind="ExternalInput")
    t_idx = nc.dram_tensor("idxf", [1, 128 * 8 * SDT], i16, kind="ExternalInput")
    t_wbl, t_wbr, t_bxr, t_invs, t_obias, t_magic = [], [], [], [], [], []
    for l in range(3):
        din, dout = DIMS[l]
        if l < 2:
            t_wbl.append([nc.dram_tensor(f"wbl{l}", [7 * din, 448], f32,
                                         kind="ExternalInput")])
            t_wbr.append([nc.dram_tensor(f"wbr{l}", [7 * din, 448], f32,
                                         kind="ExternalInput")])
        else:
            t_wbl.append([nc.dram_tensor("wbl2a", [128, 256], f32,
                                         kind="ExternalInput"),
                          nc.dram_tensor("wbl2b", [96, 192], f32,
                                         kind="ExternalInput")])
            t_wbr.append([nc.dram_tensor("wbr2a", [128, 256], f32,
                                         kind="ExternalInput"),
                          nc.dram_tensor("wbr2b", [96, 192], f32,
                                         kind="ExternalInput")])
        t_bxr.append(nc.dram_tensor(f"bxr{l}", [1, 64], f32, kind="ExternalInput"))
        t_invs.append(nc.dram_tensor(f"invs{l}", [1, 64], f32, kind="ExternalInput"))
        shape = [1, 64] if l == 2 else [dout, 1]
        t_obias.append(nc.dram_tensor(f"obias{l}", shape, f32, kind="ExternalInput"))
        t_magic.append(nc.dram_tensor(f"magic{l}", [1, 64], f32, kind="ExternalInput"))
    t_out = nc.dram_tensor("out", [PSH, 64], f32, kind="ExternalOutput")

    qctr = [0]

    def nextq():
        q = qctr[0] % 4
        qctr[0] += 1
        return q

    with tile.TileContext(nc) as tc:
        with (tc.tile_pool(name="const", bufs=1) as cpool,
              tc.tile_pool(name="resident", bufs=1) as rpool,
              tc.tile_pool(name="dram", bufs=1, space="DRAM") as dpool,
              tc.tile_pool(name="work", bufs=2) as wpool,
              tc.tile_pool(name="upool", bufs=3) as upool,
              tc.tile_pool(name="feed", bufs=4) as fpool,
              tc.tile_pool(name="small", bufs=4) as spool,
              tc.tile_pool(name="stage", bufs=3) as stpool,
              tc.tile_pool(name="psum", bufs=2, space="PSUM") as ppool,
              tc.tile_pool(name="psumT", bufs=2, space="PSUM") as ppoolT):

            t_tab = [dpool.tile([NTAB, 64], f32, name=f"tab{l}") for l in range(3)]
            t_agin = [dpool.tile([DIMS[l][1], PSH], f32,
                                 name=f"agin{l}") for l in range(2)]
            t_agout = [dpool.tile([NCORES, DIMS[l][1], PSH], f32,
                                  addr_space="Shared", name=f"agout{l}")
                       for l in range(2)]

            ident = cpool.tile([128, 128], f32)
            masks.make_identity(nc, ident[:, :])
            ones_row = cpool.tile([1, 128], f32)
            nc.vector.memset(ones_row[:, :], 1.0)

            def replicate_row(src_row, name):
                ps = ppoolT.tile([128, 64], f32, tag="repl")
                nc.tensor.matmul(ps[:, :], ones_row[:, :], src_row[:, :])
                rep = cpool.tile([128, 64], f32, name=name)
                nc.scalar.activation(rep[:, :], ps[:, :], AF.Copy)
                return rep

            c_bxr, c_invs, c_obias, c_W = [], [], [], []
            for l in range(3):
                din = DIMS[l][0]
                r = cpool.tile([1, 64], f32, name=f"r1_{l}")
                nc.sync.dma_start(r[:, :], t_bxr[l][:, :])
                c_bxr.append(replicate_row(r, f"bxr_{l}"))
                r2 = cpool.tile([1, 64], f32, name=f"r2_{l}")
                nc.sync.dma_start(r2[:, :], t_invs[l][:, :])
                c_invs.append(replicate_row(r2, f"invs_{l}"))
                if l == 2:
                    r3 = cpool.tile([1, 64], f32, name=f"r3_{l}")
                    nc.sync.dma_start(r3[:, :], t_obias[l][:, :])
                    c_obias.append(replicate_row(r3, f"obias_{l}"))
                else:
                    col = cpool.tile([DIMS[l][1], 1], f32, name=f"obias_{l}")
                    nc.sync.dma_start(col[:, :], t_obias[l][:, :])
                    c_obias.append(col)
                mg = cpool.tile([1, 64], f32, name=f"mg_{l}")
                nc.sync.dma_start(mg[:, :], t_magic[l][:, :])
                for ch in range(NCHUNK):
                    row = ch * CSTRIDE + CNODES
                    nc.sync.dma_start(t_tab[l][row:row + 1, :], mg[:, :])
                wls, wrs = [], []
                for i, t in enumerate(t_wbl[l]):
                    w = cpool.tile(list(t.shape), f32, name=f"cwbl{l}_{i}")
                    nc.sync.dma_start(w[:, :], t[:, :])
                    wls.append(w)
                for i, t in enumerate(t_wbr[l]):
                    w = cpool.tile(list(t.shape), f32, name=f"cwbr{l}_{i}")
                    nc.sync.dma_start(w[:, :], t[:, :])
                    wrs.append(w)
                c_W.append((wls, wrs))

            xr_res = rpool.tile([128, NBLK * 64], f32)

            def dense_group(l, din, wtiles, ps, src3d):
                """src3d: DRAM view [din, 896] of input features for 896 nodes.
                Computes ps[128, 448] = per-node-tile features (7 x 64 cols)."""
                if l < 2:
                    X = fpool.tile([7 * din, 128], f32, tag="hsl")
                    for j in range(7):
                        nc.sync.dma_start(X[j * din:(j + 1) * din, :],
                                          src3d[:, j * 128:(j + 1) * 128])
                    nc.tensor.matmul(ps[:, 0:448], X[:, :], wtiles[0][:, :])
                else:
                    X4 = fpool.tile([128, 128], f32, tag="hsl4")
                    for j in range(4):
                        nc.sync.dma_start(X4[j * din:(j + 1) * din, :],
                                          src3d[:, j * 128:(j + 1) * 128])
                    X3 = fpool.tile([96, 128], f32, tag="hsl3")
                    for j in range(3):
                        nc.sync.dma_start(X3[j * din:(j + 1) * din, :],
                                          src3d[:, (4 + j) * 128:(5 + j) * 128])
                    nc.tensor.matmul(ps[:, 0:256], X4[:, :], wtiles[0][:, :])
                    nc.tensor.matmul(ps[:, 256:448], X3[:, :], wtiles[1][:, :])

            for l in range(3):
                din, dout = DIMS[l]
                kpos = kpos_list[l]
                wls, wrs = c_W[l]

                # ---- dense: xl'' table for all nodes ----
                for G in range(NCORES * NGRP):
                    shard, g = divmod(G, NGRP)
                    if l == 0:
                        src = t_xT[:, shard * PSH + g * GRP:
                                   shard * PSH + (g + 1) * GRP]
                    else:
                        src = t_agout[l - 1][shard, :, g * GRP:(g + 1) * GRP]
                    ps = ppool.tile([128, 448], f32, tag="psd")
                    dense_group(l, din, wls, ps, src)
                    sb = wpool.tile([128, 448], f32, tag="sbd")
                    nc.scalar.activation(sb[:, :], ps[:, :], AF.Copy)
                    row0 = (shard // 2) * CSTRIDE + (shard % 2) * PSH + g * GRP
                    nc.sync.dma_start(
                        t_tab[l][row0:row0 + GRP, :].rearrange(
                            "(j p) k -> p j k", p=128),
                        sb.rearrange("p (j k) -> p j k", k=64))

                # ---- dense: xr'' for local shard into xr_res ----
                for g in range(NGRP):
                    if l == 0:
                        src = t_xTloc[:, g * GRP:(g + 1) * GRP]
                    else:
                        src = t_agin[l - 1][0:din, g * GRP:(g + 1) * GRP]
                    ps = ppool.tile([128, 448], f32, tag="psd")
                    dense_group(l, din, wrs, ps, src)
                    nc.vector.tensor_tensor(
                        xr_res.rearrange("p (b k) -> p b k", k=64)[:, 7 * g:7 * g + 7, :],
                        ps.rearrange("p (b k) -> p b k", k=64),
                        c_bxr[l].unsqueeze(1).broadcast_to((128, 7, 64)),
                        OP.add)

                # ---- edge phase ----
                for b in range(NBLK):
                    dt = int(Dtot[b])
                    bb = int(blk_base[b])
                    idx_t = fpool.tile([128, 8 * dt], i16, tag="idx")
                    nc.sync.dma_start(
                        idx_t[:, :],
                        t_idx[0, 128 * 8 * bb:128 * 8 * (bb + dt)].rearrange(
                            "(p f) -> p f", p=128))
                    u = upool.tile([128, dt * 64], f32, tag="u")
                    u3 = u.rearrange("p (d k) -> p d k", d=dt)
                    for ch in range(NCHUNK):
                        w = int(D[b, ch])
                        if w == 0:
                            continue
                        off = int(coloff[b, ch] - blk_base[b])
                        nc.gpsimd.dma_gather(
                            u3[:, off:off + w, :],
                            t_tab[l][ch * CSTRIDE:(ch + 1) * CSTRIDE, :],
                            idx_t[:, 8 * off:8 * (off + w)],
                            num_idxs=128 * w, num_idxs_reg=128 * w,
                            elem_size=64, single_packet=False,
                            queue_num=nextq())
                    xr_blk = xr_res[:, b * 64:(b + 1) * 64]
                    uS = u3[:, :, 0:dout]
                    nc.vector.tensor_tensor(
                        uS, uS,
                        xr_blk[:, 0:dout].unsqueeze(1).broadcast_to(
                            (128, dt, dout)),
                        OP.add)
                    lr = wpool.tile([128, dt * dout], f32, tag="lr")
                    lr3 = lr.rearrange("p (d k) -> p d k", d=dt)
                    e = spool.tile([128, dt], f32, tag="e")
                    eN = spool.tile([128, dt], f32, tag="eN")
                    if kpos > 0:
                        nc.vector.scalar_tensor_tensor(
                            lr3[:, :, 0:kpos], uS[:, :, 0:kpos], 0.2,
                            uS[:, :, 0:kpos], OP.mult, OP.max)
                        nc.vector.tensor_reduce(
                            e[:, :], lr3[:, :, 0:kpos], AX.X, OP.add)
                    if kpos < dout:
                        nc.vector.scalar_tensor_tensor(
                            lr3[:, :, kpos:dout], uS[:, :, kpos:dout], 0.2,
                            uS[:, :, kpos:dout], OP.mult, OP.max)
                        nc.vector.tensor_reduce(
                            eN[:, :], lr3[:, :, kpos:dout], AX.X, OP.add)
                    if 0 < kpos < dout:
                        nc.vector.tensor_tensor(e[:, :], e[:, :], eN[:, :],
                                                OP.subtract)
                    elif kpos == 0:
                        nc.vector.tensor_scalar_mul(e[:, :], eN[:, :], -1.0)
                    m = spool.tile([128, 1], f32, tag="m")
                    nc.vector.tensor_reduce(m[:, :], e[:, :], AX.X, OP.max)
                    negm = spool.tile([128, 1], f32, tag="negm")
                    nc.vector.tensor_scalar_mul(negm[:, :], m[:, :], -1.0)
                    p = spool.tile([128, dt], f32, tag="p")
                    nc.scalar.activation(p[:, :], e[:, :], AF.Exp,
                                         bias=negm[:, :])
                    den = spool.tile([128, 1], f32, tag="den")
                    nc.vector.tensor_reduce(den[:, :], p[:, :], AX.X, OP.add)
                    rden = spool.tile([128, 1], f32, tag="rden")
                    nc.vector.reciprocal(rden[:, :], den[:, :])
                    wg = wpool.tile([128, dt * dout], f32, tag="lr")
                    wg3 = wg.rearrange("p (d k) -> p d k", d=dt)
                    nc.vector.tensor_tensor(
                        wg3, uS,
                        p.unsqueeze(2).broadcast_to((128, dt, dout)), OP.mult)
                    outU = spool.tile([128, dout], f32, tag="outU")
                    nc.vector.tensor_reduce(outU[:, :],
                                            wg.rearrange("p (d k) -> p k d", d=dt),
                                            AX.X, OP.add)
                    o1 = spool.tile([128, dout], f32, tag="o1")
                    nc.vector.scalar_tensor_tensor(
                        o1[:, :], outU[:, :], rden[:, :], xr_blk[:, 0:dout],
                        OP.mult, OP.subtract)
                    o2 = spool.tile([128, dout], f32, tag="o2")
                    nc.vector.tensor_tensor(o2[:, :], o1[:, :],
                                            c_invs[l][:, 0:dout], OP.mult)
                    if l < 2:
                        trp = ppoolT.tile([64, 128], f32, tag="trp")
                        nc.tensor.transpose(trp[0:dout, :], o2[:, :], ident[:, :])
                        hst = stpool.tile([64, 128], f32, tag="hst")
                        nc.scalar.activation(
                            hst[0:dout, :],
                            trp[0:dout, :], AF.Relu, bias=c_obias[l][:, :])
                        nc.scalar.dma_start(
                            t_agin[l][:, b * 128:(b + 1) * 128],
                            hst[0:dout, :])
                    else:
                        o3 = spool.tile([128, 64], f32, tag="o3")
                        nc.vector.tensor_tensor(o3[:, :], o2[:, :],
                                                c_obias[l][:, :], OP.add)
                        nc.scalar.dma_start(t_out[b * 128:(b + 1) * 128, :],
                                            o3[:, :])

                if l < 2:
                    nc.gpsimd.collective_compute(
                        "AllGather", OP.bypass,
                        replica_groups=[list(range(NCORES))],
                        ins=[t_agin[l].opt()], outs=[t_agout[l].opt()])
    nc.compile()
    return nc


def _prep_inputs(inputs, meta):
    x = np.asarray(inputs["x"], np.float32)
    order = meta['order_per_core']
    xT = np.zeros((11, NN), np.float32)
    for c in range(NCORES):
        xT[:, c * PSH:c * PSH + SH] = x[order[c]].T
    per_layer = {}
    kpos_list = []
    prev_perm = None
    for li, l in enumerate([1, 2, 3]):
        din, dout = DIMS[li]
        Wl = np.asarray(inputs[f"Wl{l}"], np.float32)
        Wr = np.asarray(inputs[f"Wr{l}"], np.float32)
        bl = np.asarray(inputs[f"bl{l}"], np.float32)
        br = np.asarray(inputs[f"br{l}"], np.float32)
        att = np.asarray(inputs[f"att{l}"], np.float32)
        b_l = np.asarray(inputs[f"b{l}"], np.float32)
        perm = np.argsort(att < 0, kind='stable')
        kpos = int((att[perm] >= 0).sum())
        s = np.abs(att[perm])
        s_safe = np.where(s == 0, 1.0, s)
        if prev_perm is not None:
            Wl = Wl[prev_perm]
            Wr = Wr[prev_perm]
        Wlp = np.zeros((din, 64), np.float32)
        Wlp[:, :dout] = Wl[:, perm] * s
        Wrp = np.zeros((din, 64), np.float32)
        Wrp[:, :dout] = Wr[:, perm] * s

        def blockdiag(W):
            if li < 2:
                out = np.zeros((7 * din, 448), np.float32)
                for j in range(7):
                    out[j * din:(j + 1) * din, j * 64:(j + 1) * 64] = W
                return [out]
            a = np.zeros((128, 256), np.float32)
            for j in range(4):
                a[j * din:(j + 1) * din, j * 64:(j + 1) * 64] = W
            bm = np.zeros((96, 192), np.float32)
            for j in range(3):
                bm[j * din:(j + 1) * din, j * 64:(j + 1) * 64] = W
            return [a, bm]

        bxr = np.zeros((1, 64), np.float32)
        bxr[0, :dout] = (bl + br)[perm] * s
        invs = np.zeros((1, 64), np.float32)
        invs[0, :dout] = 1.0 / s_safe
        ob = (bl + b_l)[perm]
        if li == 2:
            obias = np.zeros((1, 64), np.float32)
            obias[0, :dout] = ob
        else:
            obias = ob.reshape(dout, 1).astype(np.float32)
        magic = np.zeros((1, 64), np.float32)
        magic[0, :dout] = np.where(np.arange(dout) < kpos, -1000.0, 1000.0)
        per_layer[li] = dict(wbl=blockdiag(Wlp), wbr=blockdiag(Wrp),
                             bxr=bxr, invs=invs, obias=obias,
                             magic=magic, perm=perm)
        kpos_list.append(kpos)
        prev_perm = perm
    return xT, per_layer, kpos_list


_CACHE = {}


def kernel(**inputs):
    global LAST_EXEC_NS
    from concourse import bass_utils

    edge_index = np.asarray(inputs["edge_index"])
    key = "prog"
    if key not in _CACHE:
        idx_flat, meta = _preprocess(edge_index)
        xT, per_layer, kpos_list = _prep_inputs(inputs, meta)
        nc = _build_program(meta, kpos_list)
        _CACHE[key] = (nc, idx_flat, meta, xT, per_layer)
    nc, idx_flat, meta, xT, per_layer = _CACHE[key]

    in_maps = []
    for c in range(NCORES):
        blk_base, Dtot = meta['blk_base'], meta['Dtot']
        parts = []
        for b in range(NBLK):
            bb, dt = int(blk_base[b]), int(Dtot[b])
            parts.append(idx_flat[c][:, 8 * bb:8 * (bb + dt)].reshape(-1))
        idx_c = np.concatenate(parts).reshape(1, -1)
        im = {"xT": xT, "xTloc": xT[:, c * PSH:(c + 1) * PSH].copy(),
              "idxf": idx_c}
        for li in range(3):
            pl = per_layer[li]
            if li < 2:
                im[f"wbl{li}"] = pl["wbl"][0]
                im[f"wbr{li}"] = pl["wbr"][0]
            else:
                im["wbl2a"], im["wbl2b"] = pl["wbl"]
                im["wbr2a"], im["wbr2b"] = pl["wbr"]
            im[f"bxr{li}"] = pl["bxr"]
            im[f"invs{li}"] = pl["invs"]
            im[f"obias{li}"] = pl["obias"]
            im[f"magic{li}"] = pl["magic"]
        in_maps.append(im)

    res = bass_utils.run_bass_kernel_spmd(
        nc, in_maps, core_ids=list(range(NCORES)), trace=TRACE)
    LAST_EXEC_NS = res.exec_time_ns

    perm3 = per_layer[2]["perm"]
    out = np.zeros((N, 64), np.float32)
    for c in range(NCORES):
        rows = res.results[c]["out"][:SH]
        out[meta['order_per_core'][c]] = rows
    final = np.empty((N, 64), np.float32)
    final[:, perm3] = out
    return final


# revision 6
# speedup vs baseline: 1.7659x; 1.7659x over previous
"""GATv2 3-layer kernel for 8 TRN2 NeuronCores (Bass/Tile).

Dst-sharded: each core owns 12500 dst nodes, replicates the tiny dense
transforms for all nodes into a local DRAM gather table, then runs a
dst-major edge phase (dma_gather of per-edge source rows, DVE softmax +
weighted sum), PE-transposes layer outputs and AllGathers them between
layers.

Perf structure:
- The per-edge table gather is SWDGE (gpsimd Q7 descriptor generation)
  bound; the 4 chunk-gathers per block are striped across the 4 SWDGE
  queues (queue q runs on Q7 core pair q) which pipelines descriptor
  generation ~3x.
- Nodes are grouped into 128-row blocks by (max, argmax, 2nd-max) of
  per-chunk degree so the per-(block,chunk) padded width (max over
  8 cores x 128 partitions) stays near the mean.
- Dense transforms batch 7 node-tiles per PE matmul using host-built
  block-diagonal weights; layer activations are stored in DRAM in an
  interleaved [(group,tile,feat), node] layout so each dense group is
  a single contiguous DMA, written contiguously by the edge phase.
- Padded slots point at per-chunk magic rows (+-1000) so exp -> 0; att
  is folded into the weights (u = |att|*(xl+xr)) with a sign-split
  min/max leaky-relu.
"""
import sys

sys.path.insert(0, "/opt/trn_rl_repo")

import numpy as np

N = 100000
NCORES = 8
SH = 12500
PSH = 12544                 # 98 * 128
NBLK = 98
NN = NCORES * PSH           # 100352
CSTRIDE = 25089             # chunk stride in table rows (incl magic row)
CNODES = 25088              # real rows per chunk (2 core shards)
MAGIC_LOCAL = CNODES
NCHUNK = 4
NTAB = NCHUNK * CSTRIDE
DIMS = [(11, 16), (16, 32), (32, 64)]
GRP = 896                   # dense-phase node group (7*128)
NGRP = PSH // GRP           # 14

TRACE = False
LAST_EXEC_NS = None


def _preprocess(edge_index):
    src = np.concatenate([edge_index[0].astype(np.int64), np.arange(N, dtype=np.int64)])
    dst = np.concatenate([edge_index[1].astype(np.int64), np.arange(N, dtype=np.int64)])
    node_owner = np.arange(N) // SH
    node_chunk = node_owner // 2

    cnt = np.zeros((N, NCHUNK), dtype=np.int32)
    np.add.at(cnt, (dst, node_chunk[src]), 1)

    localpos = np.empty(N, dtype=np.int64)
    order_per_core = []
    for c in range(NCORES):
        nodes = np.arange(c * SH, (c + 1) * SH)
        cc = cnt[nodes]
        srt = np.sort(cc, 1)[:, ::-1]
        o = np.lexsort((srt[:, 1], cc.argmax(1), srt[:, 0]))[::-1]
        nodes = nodes[o]
        order_per_core.append(nodes)
        localpos[nodes] = np.arange(SH)

    tabrow = (node_owner // 2) * CSTRIDE + (node_owner % 2) * PSH + localpos

    cntp = np.zeros((NCORES, NBLK, 128, NCHUNK), dtype=np.int32)
    for c in range(NCORES):
        cc = cnt[order_per_core[c]]
        cc = np.concatenate([cc, np.zeros((PSH - SH, NCHUNK), np.int32)], 0)
        cntp[c] = cc.reshape(NBLK, 128, NCHUNK)
    D = cntp.max(axis=(0, 2)).astype(np.int64)      # [NBLK, NCHUNK]
    Dtot = D.sum(axis=1)                            # [NBLK]
    SDT = int(Dtot.sum())

    blk_base = np.r_[0, np.cumsum(Dtot)][:-1]
    coloff = np.zeros((NBLK, NCHUNK), dtype=np.int64)
    for b in range(NBLK):
        coloff[b] = blk_base[b] + np.r_[0, np.cumsum(D[b])][:-1]

    # slot grid [core, 128, SDT], value = chunk-local table row of src
    ecore = dst // SH
    edl = localpos[dst]
    eblk, epart = edl // 128, edl % 128
    echunk = node_chunk[src]
    eval_loc = tabrow[src] - echunk * CSTRIDE
    key = ((ecore * NBLK + eblk) * 128 + epart) * NCHUNK + echunk
    eo = np.argsort(key, kind='stable')
    keys, vals = key[eo], eval_loc[eo]
    grp_start = np.r_[0, np.flatnonzero(np.diff(keys)) + 1]
    grp_len = np.diff(np.r_[grp_start, len(keys)])
    jidx = np.arange(len(keys)) - np.repeat(grp_start, grp_len)
    kc = keys % NCHUNK
    kp = (keys // NCHUNK) % 128
    kb = (keys // (NCHUNK * 128)) % NBLK
    kcore = keys // (NCHUNK * 128 * NBLK)
    slots = np.full((NCORES, 128, SDT), MAGIC_LOCAL, dtype=np.int64)
    slots[kcore, kp, coloff[kb, kc] + jidx] = vals

    # wrapped int16 per (block, chunk) call, concatenated along free dim
    idx_flat = np.empty((NCORES, 128, 8 * SDT), dtype=np.int16)
    for b in range(NBLK):
        for ch in range(NCHUNK):
            w = int(D[b, ch])
            if w == 0:
                continue
            cs = int(coloff[b, ch])
            sub = slots[:, :, cs:cs + w]                          # [NC,128,w]
            lst = sub.transpose(0, 2, 1).reshape(NCORES, w * 128)  # pos=j*128+p
            wr = lst.reshape(NCORES, w * 8, 16).transpose(0, 2, 1)
            wr = np.tile(wr, (1, 8, 1))
            idx_flat[:, :, 8 * cs:8 * (cs + w)] = wr.astype(np.int16)

    meta = dict(D=D, Dtot=Dtot, coloff=coloff, blk_base=blk_base,
                order_per_core=order_per_core, SDT=SDT)
    return idx_flat, meta


def _build_program(meta, kpos_list):
    import concourse.bass as bass
    import concourse.bacc as bacc
    import concourse.tile as tile
    import concourse.mybir as mybir
    from concourse import masks

    D, Dtot, coloff = meta['D'], meta['Dtot'], meta['coloff']
    blk_base, SDT = meta['blk_base'], meta['SDT']
    f32 = mybir.dt.float32
    i16 = mybir.dt.int16
    AF = mybir.ActivationFunctionType
    OP = mybir.AluOpType
    AX = mybir.AxisListType

    nc = bacc.Bacc("TRN2", target_bir_lowering=False, debug=False,
                   num_devices=NCORES, num_swdge_queues=4)
    # interleaved X layout: row (s*NGRP+g)*77 + j*11 + r, col n
    t_xTX = nc.dram_tensor("xTX", [NCORES * NGRP * 77, 128], f32,
                           kind="ExternalInput")
    t_xTlocX = nc.dram_tensor("xTlocX", [NGRP * 77, 128], f32,
                              kind="ExternalInput")
    t_idx = nc.dram_tensor("idxf", [1, 128 * 8 * SDT], i16, kind="ExternalInput")
    t_wbl, t_wbr, t_bxr, t_invs, t_obias, t_magic = [], [], [], [], [], []
    for l in range(3):
        din, dout = DIMS[l]
        if l < 2:
            t_wbl.append([nc.dram_tensor(f"wbl{l}", [7 * din, 448], f32,
                                         kind="ExternalInput")])
            t_wbr.append([nc.dram_tensor(f"wbr{l}", [7 * din, 448], f32,
                                         kind="ExternalInput")])
        else:
            t_wbl.append([nc.dram_tensor("wbl2a", [128, 256], f32,
                                         kind="ExternalInput"),
                          nc.dram_tensor("wbl2b", [96, 192], f32,
                                         kind="ExternalInput")])
            t_wbr.append([nc.dram_tensor("wbr2a", [128, 256], f32,
                                         kind="ExternalInput"),
                          nc.dram_tensor("wbr2b", [96, 192], f32,
                                         kind="ExternalInput")])
        t_bxr.append(nc.dram_tensor(f"bxr{l}", [1, 64], f32, kind="ExternalInput"))
        t_invs.append(nc.dram_tensor(f"invs{l}", [1, 64], f32, kind="ExternalInput"))
        shape = [1, 64] if l == 2 else [dout, 1]
        t_obias.append(nc.dram_tensor(f"obias{l}", shape, f32, kind="ExternalInput"))
        t_magic.append(nc.dram_tensor(f"magic{l}", [1, 64], f32, kind="ExternalInput"))
    t_out = nc.dram_tensor("out", [PSH, 64], f32, kind="ExternalOutput")

    qctr = [0]

    def nextq():
        q = qctr[0] % 4
        qctr[0] += 1
        return q

    with tile.TileContext(nc) as tc:
        with (tc.tile_pool(name="const", bufs=1) as cpool,
              tc.tile_pool(name="resident", bufs=1) as rpool,
              tc.tile_pool(name="dram", bufs=1, space="DRAM") as dpool,
              tc.tile_pool(name="work", bufs=2) as wpool,
              tc.tile_pool(name="upool", bufs=3) as upool,
              tc.tile_pool(name="feed", bufs=3) as fpool,
              tc.tile_pool(name="small", bufs=4) as spool,
              tc.tile_pool(name="stage", bufs=3) as stpool,
              tc.tile_pool(name="psum", bufs=2, space="PSUM") as ppool,
              tc.tile_pool(name="psumT", bufs=2, space="PSUM") as ppoolT):

            t_tab = [dpool.tile([NTAB, 64], f32, name=f"tab{l}") for l in range(3)]
            # interleaved layer activations: rows b*dout..(b+1)*dout = block b
            t_agin = [dpool.tile([NBLK * DIMS[l][1], 128], f32,
                                 name=f"agin{l}") for l in range(2)]
            t_agout = [dpool.tile([NCORES, NBLK * DIMS[l][1], 128], f32,
                                  addr_space="Shared", name=f"agout{l}")
                       for l in range(2)]

            ident = cpool.tile([128, 128], f32)
            masks.make_identity(nc, ident[:, :])
            ones_row = cpool.tile([1, 128], f32)
            nc.vector.memset(ones_row[:, :], 1.0)

            def replicate_row(src_row, name):
                ps = ppoolT.tile([128, 64], f32, tag="repl")
                nc.tensor.matmul(ps[:, :], ones_row[:, :], src_row[:, :])
                rep = cpool.tile([128, 64], f32, name=name)
                nc.scalar.activation(rep[:, :], ps[:, :], AF.Copy)
                return rep

            c_bxr, c_invs, c_obias, c_W = [], [], [], []
            for l in range(3):
                din = DIMS[l][0]
                r = cpool.tile([1, 64], f32, name=f"r1_{l}")
                nc.sync.dma_start(r[:, :], t_bxr[l][:, :])
                c_bxr.append(replicate_row(r, f"bxr_{l}"))
                r2 = cpool.tile([1, 64], f32, name=f"r2_{l}")
                nc.sync.dma_start(r2[:, :], t_invs[l][:, :])
                c_invs.append(replicate_row(r2, f"invs_{l}"))
                if l == 2:
                    r3 = cpool.tile([1, 64], f32, name=f"r3_{l}")
                    nc.sync.dma_start(r3[:, :], t_obias[l][:, :])
                    c_obias.append(replicate_row(r3, f"obias_{l}"))
                else:
                    col = cpool.tile([DIMS[l][1], 1], f32, name=f"obias_{l}")
                    nc.sync.dma_start(col[:, :], t_obias[l][:, :])
                    c_obias.append(col)
                mg = cpool.tile([1, 64], f32, name=f"mg_{l}")
                nc.sync.dma_start(mg[:, :], t_magic[l][:, :])
                for ch in range(NCHUNK):
                    row = ch * CSTRIDE + CNODES
                    nc.sync.dma_start(t_tab[l][row:row + 1, :], mg[:, :])
                wls, wrs = [], []
                for i, t in enumerate(t_wbl[l]):
                    w = cpool.tile(list(t.shape), f32, name=f"cwbl{l}_{i}")
                    nc.sync.dma_start(w[:, :], t[:, :])
                    wls.append(w)
                for i, t in enumerate(t_wbr[l]):
                    w = cpool.tile(list(t.shape), f32, name=f"cwbr{l}_{i}")
                    nc.sync.dma_start(w[:, :], t[:, :])
                    wrs.append(w)
                c_W.append((wls, wrs))

            xr_res = rpool.tile([128, NBLK * 64], f32)

            def dense_group(l, din, wtiles, ps, srcrows, base):
                """srcrows: DRAM tensor with interleaved X rows; base: row of
                this group's 7*din block. ps[128,448] gets 7x64 node-tile
                features."""
                if l < 2:
                    X = fpool.tile([7 * din, 128], f32, tag="hsl")
                    nc.sync.dma_start(X[:, :],
                                      srcrows[base:base + 7 * din, :])
                    nc.tensor.matmul(ps[:, 0:448], X[:, :], wtiles[0][:, :])
                else:
                    X4 = fpool.tile([128, 128], f32, tag="hsl4")
                    nc.sync.dma_start(X4[:, :], srcrows[base:base + 128, :])
                    X3 = fpool.tile([96, 128], f32, tag="hsl3")
                    nc.sync.dma_start(X3[:, :],
                                      srcrows[base + 128:base + 224, :])
                    nc.tensor.matmul(ps[:, 0:256], X4[:, :], wtiles[0][:, :])
                    nc.tensor.matmul(ps[:, 256:448], X3[:, :], wtiles[1][:, :])

            for l in range(3):
                din, dout = DIMS[l]
                kpos = kpos_list[l]
                wls, wrs = c_W[l]

                # ---- dense: xl'' table for all nodes ----
                for G in range(NCORES * NGRP):
                    shard, g = divmod(G, NGRP)
                    if l == 0:
                        srcrows, base = t_xTX, (shard * NGRP + g) * 77
                    else:
                        srcrows = t_agout[l - 1][shard]
                        base = g * 7 * din
                    ps = ppool.tile([128, 448], f32, tag="psd")
                    dense_group(l, din, wls, ps, srcrows, base)
                    sb = wpool.tile([128, 448], f32, tag="sbd")
                    nc.scalar.activation(sb[:, :], ps[:, :], AF.Copy)
                    row0 = (shard // 2) * CSTRIDE + (shard % 2) * PSH + g * GRP
                    nc.sync.dma_start(
                        t_tab[l][row0:row0 + GRP, :].rearrange(
                            "(j p) k -> p j k", p=128),
                        sb.rearrange("p (j k) -> p j k", k=64))

                # ---- dense: xr'' for local shard into xr_res ----
                for g in range(NGRP):
                    if l == 0:
                        srcrows, base = t_xTlocX, g * 77
                    else:
                        srcrows, base = t_agin[l - 1], g * 7 * din
                    ps = ppool.tile([128, 448], f32, tag="psd")
                    dense_group(l, din, wrs, ps, srcrows, base)
                    nc.vector.tensor_tensor(
                        xr_res.rearrange("p (b k) -> p b k", k=64)[:, 7 * g:7 * g + 7, :],
                        ps.rearrange("p (b k) -> p b k", k=64),
                        c_bxr[l].unsqueeze(1).broadcast_to((128, 7, 64)),
                        OP.add)

                # ---- edge phase ----
                for b in range(NBLK):
                    dt = int(Dtot[b])
                    bb = int(blk_base[b])
                    idx_t = fpool.tile([128, 8 * dt], i16, tag="idx")
                    nc.sync.dma_start(
                        idx_t[:, :],
                        t_idx[0, 128 * 8 * bb:128 * 8 * (bb + dt)].rearrange(
                            "(p f) -> p f", p=128))
                    xr_blk = xr_res[:, b * 64:(b + 1) * 64]
                    us = []
                    for ch in range(NCHUNK):
                        w = int(D[b, ch])
                        if w == 0:
                            us.append(None)
                            continue
                        off = int(coloff[b, ch] - blk_base[b])
                        u = upool.tile([128, w * 64], f32, tag=f"u{ch}")
                        nc.gpsimd.dma_gather(
                            u.rearrange("p (d k) -> p d k", d=w),
                            t_tab[l][ch * CSTRIDE:(ch + 1) * CSTRIDE, :],
                            idx_t[:, 8 * off:8 * (off + w)],
                            num_idxs=128 * w, num_idxs_reg=128 * w,
                            elem_size=64, single_packet=False,
                            queue_num=nextq())
                        us.append(u)
                    e = spool.tile([128, dt], f32, tag="e")
                    eN = spool.tile([128, dt], f32, tag="eN")
                    lrs = []
                    for ch in range(NCHUNK):
                        w = int(D[b, ch])
                        if w == 0:
                            lrs.append(None)
                            continue
                        off = int(coloff[b, ch] - blk_base[b])
                        ur = us[ch].rearrange("p (d k) -> p d k", d=w)[:, :, 0:dout]
                        lr = upool.tile([128, w * dout], f32, tag=f"lr{ch}")
                        lrr = lr.rearrange("p (d k) -> p d k", d=w)
                        lrs.append(lrr)
                        nc.vector.tensor_tensor(
                            ur, ur,
                            xr_blk[:, 0:dout].unsqueeze(1).broadcast_to(
                                (128, w, dout)),
                            OP.add)
                        if kpos > 0:
                            nc.vector.scalar_tensor_tensor(
                                lrr[:, :, 0:kpos], ur[:, :, 0:kpos], 0.2,
                                ur[:, :, 0:kpos], OP.mult, OP.max)
                            nc.vector.tensor_reduce(
                                e[:, off:off + w], lrr[:, :, 0:kpos],
                                AX.X, OP.add)
                        if kpos < dout:
                            nc.vector.scalar_tensor_tensor(
                                lrr[:, :, kpos:dout], ur[:, :, kpos:dout], 0.2,
                                ur[:, :, kpos:dout], OP.mult, OP.max)
                            nc.vector.tensor_reduce(
                                eN[:, off:off + w], lrr[:, :, kpos:dout],
                                AX.X, OP.add)
                    if 0 < kpos < dout:
                        nc.vector.tensor_tensor(e[:, :], e[:, :], eN[:, :],
                                                OP.subtract)
                    elif kpos == 0:
                        nc.vector.tensor_scalar_mul(e[:, :], eN[:, :], -1.0)
                    m = spool.tile([128, 1], f32, tag="m")
                    nc.vector.tensor_reduce(m[:, :], e[:, :], AX.X, OP.max)
                    negm = spool.tile([128, 1], f32, tag="negm")
                    nc.vector.tensor_scalar_mul(negm[:, :], m[:, :], -1.0)
                    p = spool.tile([128, dt], f32, tag="p")
                    nc.scalar.activation(p[:, :], e[:, :], AF.Exp,
                                         bias=negm[:, :])
                    den = spool.tile([128, 1], f32, tag="den")
                    nc.vector.tensor_reduce(den[:, :], p[:, :], AX.X, OP.add)
                    rden = spool.tile([128, 1], f32, tag="rden")
                    nc.vector.reciprocal(rden[:, :], den[:, :])
                    outU = spool.tile([128, dout], f32, tag="outU")
                    first = True
                    for ch in range(NCHUNK):
                        w = int(D[b, ch])
                        if w == 0:
                            continue
                        off = int(coloff[b, ch] - blk_base[b])
                        ur = us[ch].rearrange("p (d k) -> p d k", d=w)[:, :, 0:dout]
                        wg3 = lrs[ch]
                        nc.vector.tensor_tensor(
                            wg3, ur,
                            p[:, off:off + w].unsqueeze(2).broadcast_to(
                                (128, w, dout)), OP.mult)
                        tgt = outU if first else spool.tile([128, dout], f32,
                                                            tag="outC")
                        nc.vector.tensor_reduce(
                            tgt[:, :],
                            wg3.rearrange("p d k -> p k d"),
                            AX.X, OP.add)
                        if not first:
                            nc.vector.tensor_tensor(outU[:, :], outU[:, :],
                                                    tgt[:, :], OP.add)
                        first = False
                    o1 = spool.tile([128, dout], f32, tag="o1")
                    nc.vector.scalar_tensor_tensor(
                        o1[:, :], outU[:, :], rden[:, :], xr_blk[:, 0:dout],
                        OP.mult, OP.subtract)
                    o2 = spool.tile([128, dout], f32, tag="o2")
                    nc.vector.tensor_tensor(o2[:, :], o1[:, :],
                                            c_invs[l][:, 0:dout], OP.mult)
                    if l < 2:
                        trp = ppoolT.tile([64, 128], f32, tag="trp")
                        nc.tensor.transpose(trp[0:dout, :], o2[:, :], ident[:, :])
                        hst = stpool.tile([64, 128], f32, tag="hst")
                        nc.scalar.activation(
                            hst[0:dout, :],
                            trp[0:dout, :], AF.Relu, bias=c_obias[l][:, :])
                        nc.sync.dma_start(
                            t_agin[l][b * dout:(b + 1) * dout, :],
                            hst[0:dout, :])
                    else:
                        o3 = spool.tile([128, 64], f32, tag="o3")
                        nc.vector.tensor_tensor(o3[:, :], o2[:, :],
                                                c_obias[l][:, :], OP.add)
                        nc.sync.dma_start(t_out[b * 128:(b + 1) * 128, :],
                                          o3[:, :])

                if l < 2:
                    nc.gpsimd.collective_compute(
                        "AllGather", OP.bypass,
                        replica_groups=[list(range(NCORES))],
                        ins=[t_agin[l].opt()], outs=[t_agout[l].opt()])
    nc.compile()
    return nc


def _prep_inputs(inputs, meta):
    x = np.asarray(inputs["x"], np.float32)
    order = meta['order_per_core']
    xT = np.zeros((11, NN), np.float32)
    for c in range(NCORES):
        xT[:, c * PSH:c * PSH + SH] = x[order[c]].T
    # interleaved X layout [NCORES, NGRP, 7, 11, 128] -> rows
    xTX = np.ascontiguousarray(
        xT.reshape(11, NCORES, NGRP, 7, 128).transpose(1, 2, 3, 0, 4)
    ).reshape(NCORES * NGRP * 7 * 11, 128)
    per_layer = {}
    kpos_list = []
    prev_perm = None
    for li, l in enumerate([1, 2, 3]):
        din, dout = DIMS[li]
        Wl = np.asarray(inputs[f"Wl{l}"], np.float32)
        Wr = np.asarray(inputs[f"Wr{l}"], np.float32)
        bl = np.asarray(inputs[f"bl{l}"], np.float32)
        br = np.asarray(inputs[f"br{l}"], np.float32)
        att = np.asarray(inputs[f"att{l}"], np.float32)
        b_l = np.asarray(inputs[f"b{l}"], np.float32)
        perm = np.argsort(att < 0, kind='stable')
        kpos = int((att[perm] >= 0).sum())
        s = np.abs(att[perm])
        s_safe = np.where(s == 0, 1.0, s)
        if prev_perm is not None:
            Wl = Wl[prev_perm]
            Wr = Wr[prev_perm]
        Wlp = np.zeros((din, 64), np.float32)
        Wlp[:, :dout] = Wl[:, perm] * s
        Wrp = np.zeros((din, 64), np.float32)
        Wrp[:, :dout] = Wr[:, perm] * s

        def blockdiag(W):
            if li < 2:
                out = np.zeros((7 * din, 448), np.float32)
                for j in range(7):
                    out[j * din:(j + 1) * din, j * 64:(j + 1) * 64] = W
                return [out]
            a = np.zeros((128, 256), np.float32)
            for j in range(4):
                a[j * din:(j + 1) * din, j * 64:(j + 1) * 64] = W
            bm = np.zeros((96, 192), np.float32)
            for j in range(3):
                bm[j * din:(j + 1) * din, j * 64:(j + 1) * 64] = W
            return [a, bm]

        bxr = np.zeros((1, 64), np.float32)
        bxr[0, :dout] = (bl + br)[perm] * s
        invs = np.zeros((1, 64), np.float32)
        invs[0, :dout] = 1.0 / s_safe
        ob = (bl + b_l)[perm]
        if li == 2:
            obias = np.zeros((1, 64), np.float32)
            obias[0, :dout] = ob
        else:
            obias = ob.reshape(dout, 1).astype(np.float32)
        magic = np.zeros((1, 64), np.float32)
        magic[0, :dout] = np.where(np.arange(dout) < kpos, -1000.0, 1000.0)
        per_layer[li] = dict(wbl=blockdiag(Wlp), wbr=blockdiag(Wrp),
                             bxr=bxr, invs=invs, obias=obias,
                             magic=magic, perm=perm)
        kpos_list.append(kpos)
        prev_perm = perm
    return xTX, per_layer, kpos_list


_CACHE = {}


def kernel(**inputs):
    global LAST_EXEC_NS
    from concourse import bass_utils

    edge_index = np.asarray(inputs["edge_index"])
    key = "prog"
    if key not in _CACHE:
        idx_flat, meta = _preprocess(edge_index)
        xTX, per_layer, kpos_list = _prep_inputs(inputs, meta)
        nc = _build_program(meta, kpos_list)
        _CACHE[key] = (nc, idx_flat, meta, xTX, per_layer)
    nc, idx_flat, meta, xTX, per_layer = _CACHE[key]

    rows_per_core = NGRP * 77
    in_maps = []
    for c in range(NCORES):
        blk_base, Dtot = meta['blk_base'], meta['Dtot']
        parts = []
        for b in range(NBLK):
            bb, dt = int(blk_base[b]), int(Dtot[b])
            parts.append(idx_flat[c][:, 8 * bb:8 * (bb + dt)].reshape(-1))
        idx_c = np.concatenate(parts).reshape(1, -1)
        im = {"xTX": xTX,
              "xTlocX": xTX[c * rows_per_core:(c + 1) * rows_per_core].copy(),
              "idxf": idx_c}
        for li in range(3):
            pl = per_layer[li]
            if li < 2:
                im[f"wbl{li}"] = pl["wbl"][0]
                im[f"wbr{li}"] = pl["wbr"][0]
            else:
                im["wbl2a"], im["wbl2b"] = pl["wbl"]
                im["wbr2a"], im["wbr2b"] = pl["wbr"]
            im[f"bxr{li}"] = pl["bxr"]
            im[f"invs{li}"] = pl["invs"]
            im[f"obias{li}"] = pl["obias"]
            im[f"magic{li}"] = pl["magic"]
        in_maps.append(im)

    res = bass_utils.run_bass_kernel_spmd(
        nc, in_maps, core_ids=list(range(NCORES)), trace=TRACE)
    LAST_EXEC_NS = res.exec_time_ns

    perm3 = per_layer[2]["perm"]
    out = np.zeros((N, 64), np.float32)
    for c in range(NCORES):
        rows = res.results[c]["out"][:SH]
        out[meta['order_per_core'][c]] = rows
    final = np.empty((N, 64), np.float32)
    final[:, perm3] = out
    return final


# revision 8
# speedup vs baseline: 1.8695x; 1.0587x over previous
"""GATv2 3-layer kernel for 8 TRN2 NeuronCores (Bass/Tile).

Dst-sharded: each core owns 12500 dst nodes, replicates the tiny dense
transforms for all nodes into a local DRAM gather table, then runs a
dst-major edge phase (dma_gather of per-edge source rows, DVE softmax +
weighted sum), PE-transposes layer outputs and AllGathers them between
layers.

Perf structure:
- The per-edge table gather is SWDGE (gpsimd Q7 descriptor generation)
  bound; the 4 chunk-gathers per block are striped across the 4 SWDGE
  queues (queue q runs on Q7 core pair q) which pipelines descriptor
  generation ~3x.
- Nodes are grouped into 128-row blocks by (max, argmax, 2nd-max) of
  per-chunk degree so the per-(block,chunk) padded width (max over
  8 cores x 128 partitions) stays near the mean.
- Dense transforms batch 7 node-tiles per PE matmul using host-built
  block-diagonal weights; layer activations are stored in DRAM in an
  interleaved [(group,tile,feat), node] layout so each dense group is
  a single contiguous DMA, written contiguously by the edge phase.
- Padded slots point at per-chunk magic rows (+-1000) so exp -> 0; att
  is folded into the weights (u = |att|*(xl+xr)) with a sign-split
  min/max leaky-relu.
"""
import sys

sys.path.insert(0, "/opt/trn_rl_repo")

import numpy as np

N = 100000
NCORES = 8
SH = 12500
PSH = 12544                 # 98 * 128
NBLK = 98
NN = NCORES * PSH           # 100352
CSTRIDE = 25089             # chunk stride in table rows (incl magic row)
CNODES = 25088              # real rows per chunk (2 core shards)
MAGIC_LOCAL = CNODES
NCHUNK = 4
NTAB = NCHUNK * CSTRIDE
DIMS = [(11, 16), (16, 32), (32, 64)]
GRP = 896                   # dense-phase node group (7*128)
NGRP = PSH // GRP           # 14

TRACE = False
LAST_EXEC_NS = None


def _preprocess(edge_index):
    src = np.concatenate([edge_index[0].astype(np.int64), np.arange(N, dtype=np.int64)])
    dst = np.concatenate([edge_index[1].astype(np.int64), np.arange(N, dtype=np.int64)])
    node_owner = np.arange(N) // SH
    node_chunk = node_owner // 2

    cnt = np.zeros((N, NCHUNK), dtype=np.int32)
    np.add.at(cnt, (dst, node_chunk[src]), 1)

    localpos = np.empty(N, dtype=np.int64)
    order_per_core = []
    for c in range(NCORES):
        nodes = np.arange(c * SH, (c + 1) * SH)
        cc = cnt[nodes]
        srt = np.sort(cc, 1)[:, ::-1]
        o = np.lexsort((srt[:, 1], cc.argmax(1), srt[:, 0]))[::-1]
        nodes = nodes[o]
        order_per_core.append(nodes)
        localpos[nodes] = np.arange(SH)

    tabrow = (node_owner // 2) * CSTRIDE + (node_owner % 2) * PSH + localpos

    cntp = np.zeros((NCORES, NBLK, 128, NCHUNK), dtype=np.int32)
    for c in range(NCORES):
        cc = cnt[order_per_core[c]]
        cc = np.concatenate([cc, np.zeros((PSH - SH, NCHUNK), np.int32)], 0)
        cntp[c] = cc.reshape(NBLK, 128, NCHUNK)
    D = cntp.max(axis=(0, 2)).astype(np.int64)      # [NBLK, NCHUNK]
    Dtot = D.sum(axis=1)                            # [NBLK]
    SDT = int(Dtot.sum())

    blk_base = np.r_[0, np.cumsum(Dtot)][:-1]
    coloff = np.zeros((NBLK, NCHUNK), dtype=np.int64)
    for b in range(NBLK):
        coloff[b] = blk_base[b] + np.r_[0, np.cumsum(D[b])][:-1]

    # slot grid [core, 128, SDT], value = chunk-local table row of src
    ecore = dst // SH
    edl = localpos[dst]
    eblk, epart = edl // 128, edl % 128
    echunk = node_chunk[src]
    eval_loc = tabrow[src] - echunk * CSTRIDE
    key = ((ecore * NBLK + eblk) * 128 + epart) * NCHUNK + echunk
    eo = np.argsort(key, kind='stable')
    keys, vals = key[eo], eval_loc[eo]
    grp_start = np.r_[0, np.flatnonzero(np.diff(keys)) + 1]
    grp_len = np.diff(np.r_[grp_start, len(keys)])
    jidx = np.arange(len(keys)) - np.repeat(grp_start, grp_len)
    kc = keys % NCHUNK
    kp = (keys // NCHUNK) % 128
    kb = (keys // (NCHUNK * 128)) % NBLK
    kcore = keys // (NCHUNK * 128 * NBLK)
    slots = np.full((NCORES, 128, SDT), MAGIC_LOCAL, dtype=np.int64)
    slots[kcore, kp, coloff[kb, kc] + jidx] = vals

    # wrapped int16 per (block, chunk) call, concatenated along free dim
    idx_flat = np.empty((NCORES, 128, 8 * SDT), dtype=np.int16)
    for b in range(NBLK):
        for ch in range(NCHUNK):
            w = int(D[b, ch])
            if w == 0:
                continue
            cs = int(coloff[b, ch])
            sub = slots[:, :, cs:cs + w]                          # [NC,128,w]
            lst = sub.transpose(0, 2, 1).reshape(NCORES, w * 128)  # pos=j*128+p
            wr = lst.reshape(NCORES, w * 8, 16).transpose(0, 2, 1)
            wr = np.tile(wr, (1, 8, 1))
            idx_flat[:, :, 8 * cs:8 * (cs + w)] = wr.astype(np.int16)

    meta = dict(D=D, Dtot=Dtot, coloff=coloff, blk_base=blk_base,
                order_per_core=order_per_core, SDT=SDT)
    return idx_flat, meta


def _build_program(meta, kpos_list):
    import concourse.bass as bass
    import concourse.bacc as bacc
    import concourse.tile as tile
    import concourse.mybir as mybir
    from concourse import masks

    D, Dtot, coloff = meta['D'], meta['Dtot'], meta['coloff']
    blk_base, SDT = meta['blk_base'], meta['SDT']
    f32 = mybir.dt.float32
    bf16 = mybir.dt.bfloat16
    i16 = mybir.dt.int16
    AF = mybir.ActivationFunctionType
    OP = mybir.AluOpType
    AX = mybir.AxisListType

    nc = bacc.Bacc("TRN2", target_bir_lowering=False, debug=False,
                   num_devices=NCORES, num_swdge_queues=4)
    # interleaved X layout: row (s*NGRP+g)*77 + j*11 + r, col n
    t_xTX = nc.dram_tensor("xTX", [NCORES * NGRP * 77, 128], f32,
                           kind="ExternalInput")
    t_xTlocX = nc.dram_tensor("xTlocX", [NGRP * 77, 128], f32,
                              kind="ExternalInput")
    t_idx = nc.dram_tensor("idxf", [1, 128 * 8 * SDT], i16, kind="ExternalInput")
    t_wbl, t_wbr, t_bxr, t_invs, t_obias, t_magic = [], [], [], [], [], []
    for l in range(3):
        din, dout = DIMS[l]
        if l < 2:
            t_wbl.append([nc.dram_tensor(f"wbl{l}", [7 * din, 448], f32,
                                         kind="ExternalInput")])
            t_wbr.append([nc.dram_tensor(f"wbr{l}", [7 * din, 448], f32,
                                         kind="ExternalInput")])
        else:
            t_wbl.append([nc.dram_tensor("wbl2a", [128, 256], f32,
                                         kind="ExternalInput"),
                          nc.dram_tensor("wbl2b", [96, 192], f32,
                                         kind="ExternalInput")])
            t_wbr.append([nc.dram_tensor("wbr2a", [128, 256], f32,
                                         kind="ExternalInput"),
                          nc.dram_tensor("wbr2b", [96, 192], f32,
                                         kind="ExternalInput")])
        t_bxr.append(nc.dram_tensor(f"bxr{l}", [1, 64], f32, kind="ExternalInput"))
        t_invs.append(nc.dram_tensor(f"invs{l}", [1, 64], f32, kind="ExternalInput"))
        shape = [1, 64] if l == 2 else [dout, 1]
        t_obias.append(nc.dram_tensor(f"obias{l}", shape, f32, kind="ExternalInput"))
        t_magic.append(nc.dram_tensor(f"magic{l}", [1, 128], bf16, kind="ExternalInput"))
    t_out = nc.dram_tensor("out", [PSH, 64], f32, kind="ExternalOutput")

    qctr = [0]

    def nextq():
        q = qctr[0] % 4
        qctr[0] += 1
        return q

    with tile.TileContext(nc) as tc:
        with (tc.tile_pool(name="const", bufs=1) as cpool,
              tc.tile_pool(name="resident", bufs=1) as rpool,
              tc.tile_pool(name="dram", bufs=1, space="DRAM") as dpool,
              tc.tile_pool(name="work", bufs=2) as wpool,
              tc.tile_pool(name="upool", bufs=3) as upool,
              tc.tile_pool(name="feed", bufs=3) as fpool,
              tc.tile_pool(name="small", bufs=4) as spool,
              tc.tile_pool(name="stage", bufs=3) as stpool,
              tc.tile_pool(name="psum", bufs=2, space="PSUM") as ppool,
              tc.tile_pool(name="psumT", bufs=2, space="PSUM") as ppoolT):

            t_tab = [dpool.tile([NTAB, 128], bf16, name=f"tab{l}")
                     for l in range(3)]
            # interleaved layer activations: rows b*dout..(b+1)*dout = block b
            t_agin = [dpool.tile([NBLK * DIMS[l][1], 128], f32,
                                 name=f"agin{l}") for l in range(2)]
            t_agout = [dpool.tile([NCORES, NBLK * DIMS[l][1], 128], f32,
                                  addr_space="Shared", name=f"agout{l}")
                       for l in range(2)]

            ident = cpool.tile([128, 128], f32)
            masks.make_identity(nc, ident[:, :])
            ones_row = cpool.tile([1, 128], f32)
            nc.vector.memset(ones_row[:, :], 1.0)

            def replicate_row(src_row, name):
                ps = ppoolT.tile([128, 64], f32, tag="repl")
                nc.tensor.matmul(ps[:, :], ones_row[:, :], src_row[:, :])
                rep = cpool.tile([128, 64], f32, name=name)
                nc.scalar.activation(rep[:, :], ps[:, :], AF.Copy)
                return rep

            c_bxr, c_invs, c_obias, c_W = [], [], [], []
            for l in range(3):
                din = DIMS[l][0]
                r = cpool.tile([1, 64], f32, name=f"r1_{l}")
                nc.sync.dma_start(r[:, :], t_bxr[l][:, :])
                c_bxr.append(replicate_row(r, f"bxr_{l}"))
                r2 = cpool.tile([1, 64], f32, name=f"r2_{l}")
                nc.sync.dma_start(r2[:, :], t_invs[l][:, :])
                c_invs.append(replicate_row(r2, f"invs_{l}"))
                if l == 2:
                    r3 = cpool.tile([1, 64], f32, name=f"r3_{l}")
                    nc.sync.dma_start(r3[:, :], t_obias[l][:, :])
                    c_obias.append(replicate_row(r3, f"obias_{l}"))
                else:
                    col = cpool.tile([DIMS[l][1], 1], f32, name=f"obias_{l}")
                    nc.sync.dma_start(col[:, :], t_obias[l][:, :])
                    c_obias.append(col)
                mg = cpool.tile([1, 128], bf16, name=f"mg_{l}")
                nc.sync.dma_start(mg[:, :], t_magic[l][:, :])
                for ch in range(NCHUNK):
                    row = ch * CSTRIDE + CNODES
                    nc.sync.dma_start(t_tab[l][row:row + 1, :], mg[:, :])
                wls, wrs = [], []
                for i, t in enumerate(t_wbl[l]):
                    w = cpool.tile(list(t.shape), f32, name=f"cwbl{l}_{i}")
                    nc.sync.dma_start(w[:, :], t[:, :])
                    wls.append(w)
                for i, t in enumerate(t_wbr[l]):
                    w = cpool.tile(list(t.shape), f32, name=f"cwbr{l}_{i}")
                    nc.sync.dma_start(w[:, :], t[:, :])
                    wrs.append(w)
                c_W.append((wls, wrs))

            xr_res = rpool.tile([128, NBLK * 64], f32)
            xr_resB = rpool.tile([128, NBLK * 64], bf16)

            def dense_group(l, din, wtiles, ps, srcrows, base):
                """srcrows: DRAM tensor with interleaved X rows; base: row of
                this group's 7*din block. ps[128,448] gets 7x64 node-tile
                features."""
                if l < 2:
                    X = fpool.tile([7 * din, 128], f32, tag="hsl")
                    nc.sync.dma_start(X[:, :],
                                      srcrows[base:base + 7 * din, :])
                    nc.tensor.matmul(ps[:, 0:448], X[:, :], wtiles[0][:, :])
                else:
                    X4 = fpool.tile([128, 128], f32, tag="hsl4")
                    nc.sync.dma_start(X4[:, :], srcrows[base:base + 128, :])
                    X3 = fpool.tile([96, 128], f32, tag="hsl3")
                    nc.sync.dma_start(X3[:, :],
                                      srcrows[base + 128:base + 224, :])
                    nc.tensor.matmul(ps[:, 0:256], X4[:, :], wtiles[0][:, :])
                    nc.tensor.matmul(ps[:, 256:448], X3[:, :], wtiles[1][:, :])

            for l in range(3):
                din, dout = DIMS[l]
                kpos = kpos_list[l]
                wls, wrs = c_W[l]

                # ---- dense: xl'' table for all nodes ----
                for G in range(NCORES * NGRP):
                    shard, g = divmod(G, NGRP)
                    if l == 0:
                        srcrows, base = t_xTX, (shard * NGRP + g) * 77
                    else:
                        srcrows = t_agout[l - 1][shard]
                        base = g * 7 * din
                    ps = ppool.tile([128, 448], f32, tag="psd")
                    dense_group(l, din, wls, ps, srcrows, base)
                    sb = wpool.tile([128, 448], bf16, tag="sbd")
                    nc.scalar.activation(sb[:, :], ps[:, :], AF.Copy)
                    row0 = (shard // 2) * CSTRIDE + (shard % 2) * PSH + g * GRP
                    nc.sync.dma_start(
                        t_tab[l][row0:row0 + GRP, 0:64].rearrange(
                            "(j p) k -> p j k", p=128),
                        sb.rearrange("p (j k) -> p j k", k=64))

                # ---- dense: xr'' for local shard into xr_res ----
                for g in range(NGRP):
                    if l == 0:
                        srcrows, base = t_xTlocX, g * 77
                    else:
                        srcrows, base = t_agin[l - 1], g * 7 * din
                    ps = ppool.tile([128, 448], f32, tag="psd")
                    dense_group(l, din, wrs, ps, srcrows, base)
                    nc.vector.tensor_tensor(
                        xr_res.rearrange("p (b k) -> p b k", k=64)[:, 7 * g:7 * g + 7, :],
                        ps.rearrange("p (b k) -> p b k", k=64),
                        c_bxr[l].unsqueeze(1).broadcast_to((128, 7, 64)),
                        OP.add)
                nc.scalar.activation(xr_resB[:, :], xr_res[:, :], AF.Copy)

                # ---- edge phase ----
                for b in range(NBLK):
                    dt = int(Dtot[b])
                    bb = int(blk_base[b])
                    idx_t = fpool.tile([128, 8 * dt], i16, tag="idx")
                    nc.sync.dma_start(
                        idx_t[:, :],
                        t_idx[0, 128 * 8 * bb:128 * 8 * (bb + dt)].rearrange(
                            "(p f) -> p f", p=128))
                    xr_blk = xr_res[:, b * 64:(b + 1) * 64]
                    us = []
                    for ch in range(NCHUNK):
                        w = int(D[b, ch])
                        if w == 0:
                            us.append(None)
                            continue
                        off = int(coloff[b, ch] - blk_base[b])
                        u = upool.tile([128, w * 128], bf16, tag=f"u{ch}")
                        nc.gpsimd.dma_gather(
                            u.rearrange("p (d k) -> p d k", d=w),
                            t_tab[l][ch * CSTRIDE:(ch + 1) * CSTRIDE, :],
                            idx_t[:, 8 * off:8 * (off + w)],
                            num_idxs=128 * w, num_idxs_reg=128 * w,
                            elem_size=128, single_packet=False,
                            queue_num=nextq())
                        us.append(u)
                    e = spool.tile([128, dt], f32, tag="e")
                    eN = spool.tile([128, dt], f32, tag="eN")
                    lrs = []
                    for ch in range(NCHUNK):
                        w = int(D[b, ch])
                        if w == 0:
                            lrs.append(None)
                            continue
                        off = int(coloff[b, ch] - blk_base[b])
                        ur = us[ch].rearrange("p (d k) -> p d k", d=w)[:, :, 0:dout]
                        lr = upool.tile([128, w * dout], bf16, tag=f"lr{ch}")
                        lrr = lr.rearrange("p (d k) -> p d k", d=w)
                        lrs.append(lrr)
                        nc.vector.tensor_tensor(
                            ur, ur,
                            xr_resB[:, b * 64:b * 64 + dout].unsqueeze(1)
                            .broadcast_to((128, w, dout)),
                            OP.add)
                        if kpos > 0:
                            nc.vector.scalar_tensor_tensor(
                                lrr[:, :, 0:kpos], ur[:, :, 0:kpos], 0.2,
                                ur[:, :, 0:kpos], OP.mult, OP.max)
                            nc.vector.tensor_reduce(
                                e[:, off:off + w], lrr[:, :, 0:kpos],
                                AX.X, OP.add)
                        if kpos < dout:
                            nc.vector.scalar_tensor_tensor(
                                lrr[:, :, kpos:dout], ur[:, :, kpos:dout], 0.2,
                                ur[:, :, kpos:dout], OP.mult, OP.max)
                            nc.vector.tensor_reduce(
                                eN[:, off:off + w], lrr[:, :, kpos:dout],
                                AX.X, OP.add)
                    if 0 < kpos < dout:
                        nc.vector.tensor_tensor(e[:, :], e[:, :], eN[:, :],
                                                OP.subtract)
                    elif kpos == 0:
                        nc.vector.tensor_scalar_mul(e[:, :], eN[:, :], -1.0)
                    m = spool.tile([128, 1], f32, tag="m")
                    nc.vector.tensor_reduce(m[:, :], e[:, :], AX.X, OP.max)
                    negm = spool.tile([128, 1], f32, tag="negm")
                    nc.vector.tensor_scalar_mul(negm[:, :], m[:, :], -1.0)
                    p = spool.tile([128, dt], bf16, tag="p")
                    nc.scalar.activation(p[:, :], e[:, :], AF.Exp,
                                         bias=negm[:, :])
                    den = spool.tile([128, 1], f32, tag="den")
                    nc.vector.tensor_reduce(den[:, :], p[:, :], AX.X, OP.add)
                    rden = spool.tile([128, 1], f32, tag="rden")
                    nc.vector.reciprocal(rden[:, :], den[:, :])
                    outU = spool.tile([128, dout], f32, tag="outU")
                    first = True
                    for ch in range(NCHUNK):
                        w = int(D[b, ch])
                        if w == 0:
                            continue
                        off = int(coloff[b, ch] - blk_base[b])
                        ur = us[ch].rearrange("p (d k) -> p d k", d=w)[:, :, 0:dout]
                        wg3 = lrs[ch]
                        nc.vector.tensor_tensor(
                            wg3, ur,
                            p[:, off:off + w].unsqueeze(2).broadcast_to(
                                (128, w, dout)), OP.mult)
                        tgt = outU if first else spool.tile([128, dout], f32,
                                                            tag="outC")
                        nc.vector.tensor_reduce(
                            tgt[:, :],
                            wg3.rearrange("p d k -> p k d"),
                            AX.X, OP.add)
                        if not first:
                            nc.vector.tensor_tensor(outU[:, :], outU[:, :],
                                                    tgt[:, :], OP.add)
                        first = False
                    o1 = spool.tile([128, dout], f32, tag="o1")
                    nc.vector.scalar_tensor_tensor(
                        o1[:, :], outU[:, :], rden[:, :], xr_blk[:, 0:dout],
                        OP.mult, OP.subtract)
                    o2 = spool.tile([128, dout], f32, tag="o2")
                    nc.vector.tensor_tensor(o2[:, :], o1[:, :],
                                            c_invs[l][:, 0:dout], OP.mult)
                    if l < 2:
                        trp = ppoolT.tile([64, 128], f32, tag="trp")
                        nc.tensor.transpose(trp[0:dout, :], o2[:, :], ident[:, :])
                        hst = stpool.tile([64, 128], f32, tag="hst")
                        nc.scalar.activation(
                            hst[0:dout, :],
                            trp[0:dout, :], AF.Relu, bias=c_obias[l][:, :])
                        nc.sync.dma_start(
                            t_agin[l][b * dout:(b + 1) * dout, :],
                            hst[0:dout, :])
                    else:
                        o3 = spool.tile([128, 64], f32, tag="o3")
                        nc.vector.tensor_tensor(o3[:, :], o2[:, :],
                                                c_obias[l][:, :], OP.add)
                        nc.sync.dma_start(t_out[b * 128:(b + 1) * 128, :],
                                          o3[:, :])

                if l < 2:
                    nc.gpsimd.collective_compute(
                        "AllGather", OP.bypass,
                        replica_groups=[list(range(NCORES))],
                        ins=[t_agin[l].opt()], outs=[t_agout[l].opt()])
    nc.compile()
    return nc


def _prep_inputs(inputs, meta):
    x = np.asarray(inputs["x"], np.float32)
    order = meta['order_per_core']
    xT = np.zeros((11, NN), np.float32)
    for c in range(NCORES):
        xT[:, c * PSH:c * PSH + SH] = x[order[c]].T
    # interleaved X layout [NCORES, NGRP, 7, 11, 128] -> rows
    xTX = np.ascontiguousarray(
        xT.reshape(11, NCORES, NGRP, 7, 128).transpose(1, 2, 3, 0, 4)
    ).reshape(NCORES * NGRP * 7 * 11, 128)
    per_layer = {}
    kpos_list = []
    prev_perm = None
    for li, l in enumerate([1, 2, 3]):
        din, dout = DIMS[li]
        Wl = np.asarray(inputs[f"Wl{l}"], np.float32)
        Wr = np.asarray(inputs[f"Wr{l}"], np.float32)
        bl = np.asarray(inputs[f"bl{l}"], np.float32)
        br = np.asarray(inputs[f"br{l}"], np.float32)
        att = np.asarray(inputs[f"att{l}"], np.float32)
        b_l = np.asarray(inputs[f"b{l}"], np.float32)
        perm = np.argsort(att < 0, kind='stable')
        kpos = int((att[perm] >= 0).sum())
        s = np.abs(att[perm])
        s_safe = np.where(s == 0, 1.0, s)
        if prev_perm is not None:
            Wl = Wl[prev_perm]
            Wr = Wr[prev_perm]
        Wlp = np.zeros((din, 64), np.float32)
        Wlp[:, :dout] = Wl[:, perm] * s
        Wrp = np.zeros((din, 64), np.float32)
        Wrp[:, :dout] = Wr[:, perm] * s

        def blockdiag(W):
            if li < 2:
                out = np.zeros((7 * din, 448), np.float32)
                for j in range(7):
                    out[j * din:(j + 1) * din, j * 64:(j + 1) * 64] = W
                return [out]
            a = np.zeros((128, 256), np.float32)
            for j in range(4):
                a[j * din:(j + 1) * din, j * 64:(j + 1) * 64] = W
            bm = np.zeros((96, 192), np.float32)
            for j in range(3):
                bm[j * din:(j + 1) * din, j * 64:(j + 1) * 64] = W
            return [a, bm]

        bxr = np.zeros((1, 64), np.float32)
        bxr[0, :dout] = (bl + br)[perm] * s
        invs = np.zeros((1, 64), np.float32)
        invs[0, :dout] = 1.0 / s_safe
        ob = (bl + b_l)[perm]
        if li == 2:
            obias = np.zeros((1, 64), np.float32)
            obias[0, :dout] = ob
        else:
            obias = ob.reshape(dout, 1).astype(np.float32)
        import ml_dtypes
        magic = np.zeros((1, 128), ml_dtypes.bfloat16)
        magic[0, :dout] = np.where(np.arange(dout) < kpos,
                                   -1000.0, 1000.0).astype(ml_dtypes.bfloat16)
        per_layer[li] = dict(wbl=blockdiag(Wlp), wbr=blockdiag(Wrp),
                             bxr=bxr, invs=invs, obias=obias,
                             magic=magic, perm=perm)
        kpos_list.append(kpos)
        prev_perm = perm
    return xTX, per_layer, kpos_list


_CACHE = {}


def kernel(**inputs):
    global LAST_EXEC_NS
    from concourse import bass_utils

    edge_index = np.asarray(inputs["edge_index"])
    key = "prog"
    if key not in _CACHE:
        idx_flat, meta = _preprocess(edge_index)
        xTX, per_layer, kpos_list = _prep_inputs(inputs, meta)
        nc = _build_program(meta, kpos_list)
        _CACHE[key] = (nc, idx_flat, meta, xTX, per_layer)
    nc, idx_flat, meta, xTX, per_layer = _CACHE[key]

    rows_per_core = NGRP * 77
    in_maps = []
    for c in range(NCORES):
        blk_base, Dtot = meta['blk_base'], meta['Dtot']
        parts = []
        for b in range(NBLK):
            bb, dt = int(blk_base[b]), int(Dtot[b])
            parts.append(idx_flat[c][:, 8 * bb:8 * (bb + dt)].reshape(-1))
        idx_c = np.concatenate(parts).reshape(1, -1)
        im = {"xTX": xTX,
              "xTlocX": xTX[c * rows_per_core:(c + 1) * rows_per_core].copy(),
              "idxf": idx_c}
        for li in range(3):
            pl = per_layer[li]
            if li < 2:
                im[f"wbl{li}"] = pl["wbl"][0]
                im[f"wbr{li}"] = pl["wbr"][0]
            else:
                im["wbl2a"], im["wbl2b"] = pl["wbl"]
                im["wbr2a"], im["wbr2b"] = pl["wbr"]
            im[f"bxr{li}"] = pl["bxr"]
            im[f"invs{li}"] = pl["invs"]
            im[f"obias{li}"] = pl["obias"]
            im[f"magic{li}"] = pl["magic"]
        in_maps.append(im)

    res = bass_utils.run_bass_kernel_spmd(
        nc, in_maps, core_ids=list(range(NCORES)), trace=TRACE)
    LAST_EXEC_NS = res.exec_time_ns

    perm3 = per_layer[2]["perm"]
    out = np.zeros((N, 64), np.float32)
    for c in range(NCORES):
        rows = res.results[c]["out"][:SH]
        out[meta['order_per_core'][c]] = rows
    final = np.empty((N, 64), np.float32)
    final[:, perm3] = out
    return final
